# revision 8
# baseline (speedup 1.0000x reference)
"""FAVOR+ attention (Performer) Trainium2 Bass kernel.

Sharding: token-parallel. 8 cores, core c handles batch c//2, token half c%2
(2048 tokens each). The only cross-core communication is a 1MB AllReduce of
the per-head kv/denominator statistics over core pairs {0,1},{2,3},{4,5},{6,7}.

Device-side math per core (T=2048 tokens, H=16 heads, D=64, r=256, C=1024):
  pass A:
    qk^T = Wqk @ x^T          [2048qk, T]   (x^T, Wqk^T prepared host-side)
    aug_h = [qk_h + b ; (qk_h + b)^2]  [128, T] per head  (DVE)
    phi_k = exp(aug_k.T @ Waug - ln 16)   [T, 256] per head  (Waug rows 64:128
            are -0.5 so the matmul computes proj - |k|^2/2 directly)
    phi_q^T = exp(Waug.T @ aug_q - ln 16) [256, T] per head -> spilled to DRAM
    kvT_h += [v_h, 1].T @ phi_k           [65, 256] per head (ones col -> denom)
    v comes from its own matmul in token-major layout.
  AllReduce kvT over the batch pair.
  pass B:
    kv_aug = kvT.T (PE transpose)          [256, 65] per head
    numT = kv_aug.T @ phi_q^T              [65, T] per head (row 64 = den)
    attnT_h = numT[0:64] * recip(den+1e-6) (gpsimd partition_broadcast + DVE)
    out = attnT.T @ WprojT + bproj         [T, 1024]  token-major -> HBM
"""

import math
import sys

if "/opt/trn_rl_repo" not in sys.path:
    sys.path.insert(0, "/opt/trn_rl_repo")

import numpy as np

import concourse.bacc as bacc
import concourse.mybir as mybir
import concourse.tile as tile

F32 = mybir.dt.float32
F32R = mybir.dt.float32r
EXP = mybir.ActivationFunctionType.Exp
ADD = mybir.AluOpType.add
MULT = mybir.AluOpType.mult

H = 16
D = 64
R = 256
C = 1024
QK = 2 * C  # q+k output dims
NCORES = 8
LN_SQRT_R = math.log(math.sqrt(R))  # ln 16


def _r(ap):
    return ap


def _emit(nc, tc, io, T):
    TBLK = min(512, T)
    NTB = T // TBLK
    TT = TBLK // 128  # 128-token tiles per block

    xT = io["xT"].ap()
    wqkT = io["wqkT"].ap()
    wvT = io["wvT"].ap()
    wprojT = io["wprojT"].ap()
    bqk = io["bqk"].ap()
    bvrow = io["bvrow"].ap()
    bprojrow = io["bprojrow"].ap()
    waug = io["waug"].ap()
    ident = io["ident"].ap()
    out = io["out"].ap()

    mm = nc.tensor.matmul

    with (
        tc.tile_pool(name="consts", bufs=1) as consts,
        tc.tile_pool(name="dram", bufs=1, space="DRAM") as dpool,
    ):
        # ---------------- constants / host-prepped small tensors ----------------
        ones1 = consts.tile([1, 128], F32R)
        nc.gpsimd.memset(ones1[:].bitcast(F32), 1.0)
        ebias = consts.tile([128, 1], F32)
        nc.gpsimd.memset(ebias[:], -LN_SQRT_R)
        bqk_sb = consts.tile([128, 16], F32)
        nc.sync.dma_start(bqk_sb[:], bqk[:])
        waug_sb = consts.tile([128, R], F32R)
        nc.sync.dma_start(waug_sb[:], waug[:])
        ident_sb = consts.tile([128, 128], F32)
        nc.sync.dma_start(ident_sb[:], ident[:])
        bvr_sb = consts.tile([1, C], F32R)
        nc.sync.dma_start(bvr_sb[:], bvrow[:])
        bpr_sb = consts.tile([1, C], F32R)
        nc.sync.dma_start(bpr_sb[:], bprojrow[:])

        # broadcast v-bias row to [128, C] via ones-matmul
        bvB = consts.tile([128, C], F32)
        with tc.tile_pool(name="setup_ps", bufs=2, space="PSUM") as sps:
            for jb in range(2):
                js = slice(jb * 512, (jb + 1) * 512)
                p = sps.tile([128, 512], F32)
                mm(p[:], _r(ones1[:]), _r(bvr_sb[:, js]))
                nc.vector.tensor_copy(bvB[:, js], p[:])

        # DRAM scratch
        phiq_d = dpool.tile([H, NTB, 2, 128, TBLK], F32R)
        kvin_d = dpool.tile([H, 65, R], F32)
        kvout_d = dpool.tile([H, 65, R], F32)

        # ---------------- pass A ----------------
        with (
            tc.tile_pool(name="wqk", bufs=1) as wqkp,
            tc.tile_pool(name="wv", bufs=1) as wvp,
            tc.tile_pool(name="kvst", bufs=2) as kvstp,
            tc.tile_pool(name="xt", bufs=10) as xtp,
            tc.tile_pool(name="vsb", bufs=5) as vsbp,
            tc.tile_pool(name="aug", bufs=3) as augp,
            tc.tile_pool(name="phikst", bufs=2) as phikstp,
            tc.tile_pool(name="phiqst", bufs=2) as phiqstp,
            tc.tile_pool(name="qk_ps", bufs=2, space="PSUM") as qkps,
            tc.tile_pool(name="v_ps", bufs=1, space="PSUM") as vps,
            tc.tile_pool(name="phi_ps", bufs=2, space="PSUM") as phips,
            tc.tile_pool(name="kv_ps", bufs=1, space="PSUM") as kvps,
        ):
            wqk_sb = []
            for c in range(8):
                t = wqkp.tile([128, QK], F32R, tag=f"wqk{c}", name=f"wqk{c}")
                nc.sync.dma_start(t[:], wqkT[c * 128 : (c + 1) * 128, :])
                wqk_sb.append(t)
            wv_sb = {}
            for c in range(8):
                for jb in range(2):
                    t = wvp.tile([128, 512], F32R, tag=f"wv{c}_{jb}", name=f"wv{c}_{jb}")
                    nc.sync.dma_start(
                        t[:], wvT[c * 128 : (c + 1) * 128, jb * 512 : (jb + 1) * 512]
                    )
                    wv_sb[(c, jb)] = t

            for tb in range(NTB):
                ts = slice(tb * TBLK, (tb + 1) * TBLK)
                xts = []
                for c in range(8):
                    t = xtp.tile([128, TBLK], F32R, tag="xts", name="xts")
                    nc.sync.dma_start(t[:], xT[c * 128 : (c + 1) * 128, ts])
                    xts.append(t)

                # ---- v in token-major layout, heads strided by 68 (col 64 = 1.0)
                vt = []
                for tt in range(TT):
                    v_tile = vsbp.tile([128, H * 68], F32R, tag="vtile", name="vtile")
                    nc.gpsimd.memset(v_tile[:].bitcast(F32), 1.0)
                    for jb in range(2):
                        pv = vps.tile([128, 512], F32)
                        for c in range(8):
                            mm(
                                pv[:],
                                _r(xts[c][:, tt * 128 : (tt + 1) * 128]),
                                _r(wv_sb[(c, jb)][:]),
                                start=(c == 0),
                                stop=(c == 7),
                            )
                        dst = v_tile[:, jb * 8 * 68 : (jb + 1) * 8 * 68].rearrange(
                            "p (h c) -> p h c", c=68
                        )[:, :, 0:64]
                        src = pv[:].rearrange("p (h c) -> p h c", c=64)
                        bias = bvB[:, jb * 512 : (jb + 1) * 512].rearrange(
                            "p (h c) -> p h c", c=64
                        )
                        nc.vector.tensor_tensor(out=dst, in0=src, in1=bias, op=ADD)
                    vt.append(v_tile)

                # ---- qk -> aug -> phi -> kv/phiq
                for m in range(16):
                    pqk = qkps.tile([128, TBLK], F32)
                    for c in range(8):
                        mm(
                            pqk[:],
                            _r(wqk_sb[c][:, m * 128 : (m + 1) * 128]),
                            _r(xts[c][:]),
                            start=(c == 0),
                            stop=(c == 7),
                        )
                    augE = augp.tile([128, TBLK], F32R, tag="augE")
                    augO = augp.tile([128, TBLK], F32R, tag="augO")
                    nc.vector.tensor_scalar_add(
                        augE[0:64, :], pqk[0:64, :], bqk_sb[0:64, m : m + 1]
                    )
                    nc.vector.tensor_scalar_add(
                        augO[0:64, :], pqk[64:128, :], bqk_sb[64:128, m : m + 1]
                    )
                    nc.vector.tensor_tensor(
                        out=augE[64:128, :], in0=augE[0:64, :], in1=augE[0:64, :], op=MULT
                    )
                    nc.vector.tensor_tensor(
                        out=augO[64:128, :], in0=augO[0:64, :], in1=augO[0:64, :], op=MULT
                    )
                    for idx, aug in ((0, augE), (1, augO)):
                        if m < 8:
                            # q heads: phi_q^T [2*128r, TBLK] -> exp -> DRAM
                            h = 2 * m + idx
                            pphi = phips.tile([128, 2 * TBLK], F32)
                            for rh in range(2):
                                mm(
                                    pphi[:, rh * TBLK : (rh + 1) * TBLK],
                                    _r(waug_sb[:, rh * 128 : (rh + 1) * 128]),
                                    _r(aug[:]),
                                )
                            st = phiqstp.tile([128, 2 * TBLK], F32R, tag="phiqst")
                            nc.scalar.activation(
                                st[:], pphi[:], EXP, bias=ebias[:], scale=1.0
                            )
                            nc.sync.dma_start(
                                phiq_d[h, tb].rearrange("a p f -> p a f"),
                                st[:].rearrange("p (a f) -> p a f", a=2),
                            )
                        else:
                            # k heads: phi_k [TBLK, 256] per tt -> kv accumulation
                            h = 2 * (m - 8) + idx
                            pphi = phips.tile([128, TT * 256], F32)
                            for tt in range(TT):
                                mm(
                                    pphi[:, tt * 256 : (tt + 1) * 256],
                                    _r(aug[:, tt * 128 : (tt + 1) * 128]),
                                    _r(waug_sb[:]),
                                )
                            phik = phikstp.tile([128, TT * 256], F32R, tag="phikst")
                            nc.scalar.activation(
                                phik[:], pphi[:], EXP, bias=ebias[:], scale=1.0
                            )
                            pkv = kvps.tile([65, R], F32)
                            for tt in range(TT):
                                mm(
                                    pkv[:],
                                    _r(vt[tt][:, h * 68 : h * 68 + 65]),
                                    _r(phik[:, tt * 256 : (tt + 1) * 256]),
                                    start=(tt == 0),
                                    stop=(tt == TT - 1),
                                )
                            kvst = kvstp.tile([65, R], F32, tag="kvst", name="kvst")
                            nc.vector.tensor_copy(kvst[:], pkv[:])
                            nc.gpsimd.dma_start(
                                kvin_d[h],
                                kvst[:],
                                accum_op=(ADD if tb > 0 else mybir.AluOpType.bypass),
                            )

        # ---------------- kv AllReduce over batch pairs ----------------
        nc.gpsimd.collective_compute(
            "AllReduce",
            ADD,
            replica_groups=[[0, 1], [2, 3], [4, 5], [6, 7]],
            ins=[kvin_d[:].opt()],
            outs=[kvout_d[:].opt()],
        )

        # ---------------- pass B ----------------
        with (
            tc.tile_pool(name="wproj", bufs=1) as wprojp,
            tc.tile_pool(name="kvr", bufs=3) as kvrp,
            tc.tile_pool(name="kvaug", bufs=1) as kvaugp,
            tc.tile_pool(name="phqin", bufs=6) as phqp,
            tc.tile_pool(name="den", bufs=4) as denp,
            tc.tile_pool(name="rden", bufs=4) as rdenp,
            tc.tile_pool(name="attnT", bufs=10) as atp,
            tc.tile_pool(name="outsb", bufs=4) as outp,
            tc.tile_pool(name="tp_ps", bufs=2, space="PSUM") as tps,
            tc.tile_pool(name="num_ps", bufs=2, space="PSUM") as numps,
            tc.tile_pool(name="proj_ps", bufs=2, space="PSUM") as projps,
        ):
            bprojB = wprojp.tile([128, C], F32, tag="bprojB", name="bprojB")
            for jb in range(2):
                js = slice(jb * 512, (jb + 1) * 512)
                p = projps.tile([128, 512], F32)
                mm(p[:], _r(ones1[:]), _r(bpr_sb[:, js]))
                nc.vector.tensor_copy(bprojB[:, js], p[:])
            wproj_sb = []
            for c in range(8):
                t = wprojp.tile([128, C], F32R, tag=f"wproj{c}", name=f"wproj{c}")
                nc.sync.dma_start(t[:], wprojT[c * 128 : (c + 1) * 128, :])
                wproj_sb.append(t)

            # kv^T -> kv_aug [128r, 65] per (h, r-half) via PE transpose
            kvaug = {}
            for h in range(H):
                kvrt = kvrp.tile([65, R], F32, tag="kvr")
                nc.sync.dma_start(kvrt[:], kvout_d[h])
                for rh in range(2):
                    pt = tps.tile([128, 65], F32)
                    nc.tensor.transpose(
                        pt[:], kvrt[0:65, rh * 128 : (rh + 1) * 128], ident_sb[0:65, 0:65]
                    )
                    ka = kvaugp.tile([128, 65], F32R, tag=f"kvaug{h}_{rh}", name=f"kvaug{h}_{rh}")
                    nc.vector.tensor_copy(ka[:], pt[:])
                    kvaug[(h, rh)] = ka

            for tb in range(NTB):
                attnT = []
                for ct in range(8):
                    attnT.append(atp.tile([128, TBLK], F32R, tag="attnT", name="attnT"))
                for h in range(H):
                    phq = phqp.tile([128, 2 * TBLK], F32R, tag="phq")
                    nc.sync.dma_start(
                        phq[:].rearrange("p (a f) -> p a f", a=2),
                        phiq_d[h, tb].rearrange("a p f -> p a f"),
                    )
                    pn = numps.tile([65, TBLK], F32)
                    for rh in range(2):
                        mm(
                            pn[:],
                            _r(kvaug[(h, rh)][:]),
                            _r(phq[:, rh * TBLK : (rh + 1) * TBLK]),
                            start=(rh == 0),
                            stop=(rh == 1),
                        )
                    den = denp.tile([1, TBLK], F32, tag="den")
                    nc.vector.tensor_scalar_add(den[:], pn[64:65, :], 1e-6)
                    nc.vector.reciprocal(den[:], den[:])
                    rb = rdenp.tile([64, TBLK], F32, tag="rden")
                    nc.gpsimd.partition_broadcast(rb[:], den[:])
                    ct, half = h // 2, h % 2
                    nc.vector.tensor_tensor(
                        out=attnT[ct][64 * half : 64 * (half + 1), :],
                        in0=pn[0:64, :],
                        in1=rb[:],
                        op=MULT,
                    )
                # proj: out[t, j] token-major
                for tt in range(TT):
                    for jb in range(2):
                        pp = projps.tile([128, 512], F32)
                        for c in range(8):
                            mm(
                                pp[:],
                                _r(attnT[c][:, tt * 128 : (tt + 1) * 128]),
                                _r(wproj_sb[c][:, jb * 512 : (jb + 1) * 512]),
                                start=(c == 0),
                                stop=(c == 7),
                            )
                        ot = outp.tile([128, 512], F32, tag="outsb")
                        nc.vector.tensor_tensor(
                            out=ot[:],
                            in0=pp[:],
                            in1=bprojB[:, jb * 512 : (jb + 1) * 512],
                            op=ADD,
                        )
                        row0 = tb * TBLK + tt * 128
                        nc.sync.dma_start(
                            out[row0 : row0 + 128, jb * 512 : (jb + 1) * 512], ot[:]
                        )


def build_program(T):
    nc = bacc.Bacc(
        "TRN2", target_bir_lowering=False, debug=False, num_devices=NCORES
    )
    io = {
        "xT": nc.dram_tensor("xT", [C, T], F32R, kind="ExternalInput"),
        "wqkT": nc.dram_tensor("wqkT", [C, QK], F32R, kind="ExternalInput"),
        "wvT": nc.dram_tensor("wvT", [C, C], F32R, kind="ExternalInput"),
        "wprojT": nc.dram_tensor("wprojT", [C, C], F32R, kind="ExternalInput"),
        "bqk": nc.dram_tensor("bqk", [128, 16], F32, kind="ExternalInput"),
        "bvrow": nc.dram_tensor("bvrow", [1, C], F32R, kind="ExternalInput"),
        "bprojrow": nc.dram_tensor("bprojrow", [1, C], F32R, kind="ExternalInput"),
        "waug": nc.dram_tensor("waug", [128, R], F32R, kind="ExternalInput"),
        "ident": nc.dram_tensor("ident", [128, 128], F32, kind="ExternalInput"),
        "out": nc.dram_tensor("out", [T, C], F32, kind="ExternalOutput"),
    }
    with tile.TileContext(nc) as tc:
        _emit(nc, tc, io, T)
    nc.compile()
    return nc


def host_prep(x, Wqkv, bqkv, Wproj, bproj, random_matrix, ncores=NCORES):
    """Build the per-core input maps (all host-side numpy, outside HW timing)."""
    x = np.asarray(x, dtype=np.float32)
    Wqkv = np.asarray(Wqkv, dtype=np.float32)
    bqkv = np.asarray(bqkv, dtype=np.float32)
    Wproj = np.asarray(Wproj, dtype=np.float32)
    bproj = np.asarray(bproj, dtype=np.float32)
    rm = np.asarray(random_matrix, dtype=np.float32)

    B, N, _ = x.shape
    T = B * N // ncores
    halves = N // T if N >= T else 1

    shared = {
        "wqkT": np.ascontiguousarray(Wqkv[:QK].T),
        "wvT": np.ascontiguousarray(Wqkv[QK:].T),
        "wprojT": np.ascontiguousarray(Wproj.T),
        "bqk": np.ascontiguousarray(bqkv[:QK].reshape(16, 128).T),
        "bvrow": np.ascontiguousarray(bqkv[QK:].reshape(1, C)),
        "bprojrow": np.ascontiguousarray(bproj.reshape(1, C)),
        "waug": np.concatenate(
            [rm.T, np.full((64, R), -0.5, np.float32)], axis=0
        ).astype(np.float32),
        "ident": np.eye(128, dtype=np.float32),
    }
    in_maps = []
    for core in range(ncores):
        b = core // halves
        half = core % halves
        rows = x[b, half * T : (half + 1) * T, :]
        m = dict(shared)
        m["xT"] = np.ascontiguousarray(rows.T)
        in_maps.append(m)
    return in_maps, T


_PROGRAM_CACHE = {}


def kernel(x, Wqkv, bqkv, Wproj, bproj, random_matrix):
    from concourse.bass_utils import run_bass_kernel_spmd

    in_maps, T = host_prep(x, Wqkv, bqkv, Wproj, bproj, random_matrix)
    if T not in _PROGRAM_CACHE:
        _PROGRAM_CACHE[T] = build_program(T)
    nc = _PROGRAM_CACHE[T]
    res = run_bass_kernel_spmd(nc, in_maps, list(range(NCORES)))
    B, N, _ = np.asarray(x).shape
    halves = max(1, N // T)
    out = np.empty((B, N, C), dtype=np.float32)
    for core in range(NCORES):
        b = core // halves
        half = core % halves
        out[b, half * T : (half + 1) * T, :] = res.results[core]["out"]
    return out
